# revision 10
# baseline (speedup 1.0000x reference)
"""Trainium2 Bass kernel for NonLinearSelfAttention.

Computes, per batch b (one batch per NeuronCore, 8 cores):
    S    = x_b @ x_b.T * C**-0.5          [N, N]
    P    = softmax(S, axis=-1)
    out  = (P @ x_b) @ W.T + bias         [N, OUT]

Per-core algorithm (N=4096, C=128):
  - z = x @ W.T + bias folds the whole Linear through the attention:
    P (z + 1 bias^T) = P x W.T + bias exactly, because softmax rows sum to 1
    (the masked-diagonal term is re-added so the weights sum to r_i/r_i).
  - AV is computed TRANSPOSED with z as the stationary operand and
    E = exp(scale*S + shift) as the moving operand, in fp8 DoubleRow mode:
        outT[o, i] = sum_j z[j, o] * E[j, i]
    Each DoubleRow matmul contracts a 256-row j-pair at FD=512 (2 MACs/cell/
    cycle), unlike the [o]=129-wide standard orientation which is LDW-bound.
  - E tiles come out of the S matmul directly in the [j-part, i-col] layout
    the AV needs (S is symmetric); exp runs on the scalar engine with
    accum_out producing the softmax row sums r.
  - E is stored as fp8e5 (e5m2): its 57344 max makes overflow impossible for
    off-diagonal logits; the 2-bit mantissa noise averages out over 4096 j.
  - The diagonal (logits ~ sqrt(C), would dwarf fp8) is masked before exp by
    accumulating -30000*I into the diagonal 128x128 S block via an identity
    matmul; the exact diagonal term exp(d_i*scale+shift)*z_i (d = rowsum x^2)
    is re-added in the epilogue.
  - i is processed in two passes of 2048: outT holds 4 PSUM banks, S chunks
    double-buffer [128,1024] in the other 4.
"""
import numpy as np

import concourse.bass as bass
import concourse.tile as tile
from concourse.masks import make_identity
from concourse import bacc, mybir
from concourse import bass_utils

B = 8          # batches = cores
N = 4096       # sequence length
C = 128        # feature dim
NT = N // 128  # 32 j-tiles
WPASS = 2048   # i-columns per pass
NPASS = N // WPASS
CHUNK = 1024   # s_ps chunk width (2 PSUM banks)
SCALE = float(C) ** -0.5
SHIFT = -1.5   # exp bias; cancels in softmax
BIG = 30000.0  # diagonal mask

F32 = mybir.dt.float32
BF16 = mybir.dt.bfloat16
F8S = mybir.dt.float8e4   # stationary z
F8M = mybir.dt.float8e5   # moving E (wide range, no overflow)
DR = mybir.MatmulPerfMode.DoubleRow
EXP = mybir.ActivationFunctionType.Exp
COPY = mybir.ActivationFunctionType.Copy
MULT = mybir.AluOpType.mult
ADD = mybir.AluOpType.add


def _build(debug=False):
    nc = bacc.Bacc("TRN2", target_bir_lowering=False, debug=False, num_devices=B)
    x_d = nc.dram_tensor("x", [N, C], F32, kind="ExternalInput").ap()
    w_d = nc.dram_tensor("W", [C, C], F32, kind="ExternalInput").ap()
    b_d = nc.dram_tensor("b", [C], F32, kind="ExternalInput").ap()
    o_d = nc.dram_tensor("out", [N, C], F32, kind="ExternalOutput").ap()
    if debug:
        dbg_r = nc.dram_tensor("dbg_r", [128, 2 * NPASS, NT], F32, kind="ExternalOutput").ap()
        dbg_ed = nc.dram_tensor("dbg_ed", [128, NT], F32, kind="ExternalOutput").ap()
        dbg_rinv = nc.dram_tensor("dbg_rinv", [128, NT], F32, kind="ExternalOutput").ap()
        dbg_ot = nc.dram_tensor("dbg_ot", [128, NPASS, WPASS], F32, kind="ExternalOutput").ap()
        dbg_eb = nc.dram_tensor("dbg_eb", [128, 2, WPASS], F32, kind="ExternalOutput").ap()
        dbg_d = nc.dram_tensor("dbg_d", [128, NT], F32, kind="ExternalOutput").ap()

    with tile.TileContext(nc) as tc:
        with tc.tile_pool(name="const", bufs=1) as const, \
             tc.tile_pool(name="ebuf", bufs=2) as ebuf, \
             tc.tile_pool(name="ywork", bufs=3) as ywork, \
             tc.tile_pool(name="sps", bufs=2, space="PSUM") as sps, \
             tc.tile_pool(name="otps", bufs=1, space="PSUM") as otps:

            # ---- constants / input DMAs ----
            w_sb = const.tile([128, 128], F32)            # W [o, c]
            nc.sync.dma_start(w_sb, w_d)
            bias_bc = const.tile([128, 128], F32)         # bias broadcast
            nc.sync.dma_start(bias_bc, bass.AP(tensor=b_d.tensor, offset=b_d.offset,
                                               ap=[[0, 128]] + b_d.ap))

            x_nat = const.tile([128, NT, 128], F32)       # x [j within tile, c]
            x_view = x_d.rearrange("(t p) c -> p t c", p=128)
            bounds = [0, 2, 4, 8, 12, 16, 24, NT]
            x_bf = const.tile([128, NT, 128], BF16)
            for lo, hi in zip(bounds, bounds[1:]):
                nc.sync.dma_start(x_nat[:, lo:hi, :], x_view[:, lo:hi, :])
                nc.vector.tensor_copy(x_bf[:, lo:hi, :], x_nat[:, lo:hi, :])

            ident = const.tile([128, 128], BF16)
            make_identity(nc, ident)
            identneg = const.tile([128, 128], BF16)       # -BIG * I
            nc.scalar.mul(identneg, ident, -BIG)
            shift_b = const.tile([128, 1], F32)
            nc.vector.memset(shift_b, SHIFT)

            w_bf = const.tile([128, 128], BF16)
            nc.vector.tensor_copy(w_bf, w_sb)
            xT = const.tile([128, N], BF16)               # [c, n]

            def emit_xT_group(g):
                t_ps = sps.tile([128, 512], BF16, name="t_ps", tag="sps")
                for u in range(4):
                    nc.tensor.transpose(t_ps[:, u * 128:(u + 1) * 128],
                                        x_bf[:, g * 4 + u, :], ident)
                nc.vector.tensor_copy(xT[:, g * 512:(g + 1) * 512], t_ps)

            xT_state = {"done": 0}

            def ensure_xT(gmax):
                while xT_state["done"] < min(gmax, NT // 4):
                    emit_xT_group(xT_state["done"])
                    xT_state["done"] += 1

            ensure_xT(1)
            wt_ps = sps.tile([128, 512], BF16, name="t_ps", tag="sps")
            nc.tensor.transpose(wt_ps[:, 0:128], w_bf, ident)
            wT = const.tile([128, 128], BF16)             # wT[c, o]
            nc.vector.tensor_copy(wT, wt_ps[:, 0:128])

            # z~ = x @ W.T + bias : fp8 for the DR stationary operand, bf16
            # for the exact diagonal term in the epilogue
            z_f8 = const.tile([128, NT, 128], F8S)
            z_bf = const.tile([128, NT, 128], BF16)

            def emit_z_group(g):
                ensure_xT(g + 1)
                z_ps = sps.tile([128, 512], F32, name="z_ps", tag="sps")
                for u in range(4):
                    j = g * 4 + u
                    nc.tensor.matmul(z_ps[:, u * 128:(u + 1) * 128],
                                     xT[:, j * 128:(j + 1) * 128], wT,
                                     start=True, stop=True)
                zv = z_ps.rearrange("p (j c) -> p j c", c=128)
                zb = ywork.tile([128, 4, 128], F32, name="zb", tag="zb")
                for u in range(4):
                    nc.vector.tensor_tensor(zb[:, u, :], zv[:, u, :], bias_bc, ADD)
                nc.vector.tensor_copy(z_f8[:, g * 4:(g + 1) * 4, :], zb)
                nc.vector.tensor_copy(z_bf[:, g * 4:(g + 1) * 4, :], zb)

            z_state = {"done": 0}

            def ensure_z(gmax):
                while z_state["done"] < min(gmax, NT // 4):
                    emit_z_group(z_state["done"])
                    z_state["done"] += 1

            r_parts = const.tile([128, 2 * NPASS, NT], F32)
            ot_sb = const.tile([128, NPASS, WPASS], BF16)  # flushed outT halves

            # ---- main loop ----
            NC_ = WPASS // CHUNK                          # chunks per pass (2)
            NQ_ = WPASS // 512                            # AV quads per pass (4)

            def emit_pair_S_exp(p, m):
                eb = ebuf.tile([128, 2, WPASS], F8M, name="eb", tag="eb")
                for kt in range(2):
                    t = 2 * m + kt
                    ensure_xT(max((t + 4) // 4, (p + 1) * 4))
                    for c in range(NC_):
                        s_ps = sps.tile([128, CHUNK], F32, name="s_ps", tag="sps")
                        for u in range(CHUNK // 512):
                            i0 = p * WPASS + c * CHUNK + u * 512
                            diag = i0 <= t * 128 < i0 + 512
                            nc.tensor.matmul(s_ps[:, u * 512:(u + 1) * 512],
                                             xT[:, t * 128:(t + 1) * 128],
                                             xT[:, i0:i0 + 512],
                                             start=True, stop=True)
                            if diag:
                                off = t * 128 - i0 + u * 512
                                nc.tensor.matmul(s_ps[:, off:off + 128],
                                                 ident, identneg,
                                                 start=False, stop=True,
                                                 skip_group_check=True)
                        cg = p * NC_ + c
                        nc.scalar.activation(
                            eb[:, kt, c * CHUNK:(c + 1) * CHUNK], s_ps, EXP,
                            bias=shift_b, scale=SCALE,
                            accum_out=r_parts[:, cg, t:t + 1])
                return eb

            for p in range(NPASS):
                ot = otps.tile([128, WPASS], F32, name="ot", tag="ot")
                ebs = {}
                ebs[0] = emit_pair_S_exp(p, 0)
                if debug and p == 0:
                    ebf = const.tile([128, 2, WPASS], F32, name="ebf")
                    nc.vector.tensor_copy(ebf, ebs[0])
                    nc.sync.dma_start(dbg_eb, ebf)

                def emit_AV(m):
                    ensure_z((2 * m + 2 + 3) // 4)
                    eb = ebs.pop(m)
                    for q in range(NQ_):
                        nc.tensor.matmul(ot[:, q * 512:(q + 1) * 512],
                                         z_f8[:, 2 * m:2 * m + 2, :],
                                         eb[:, :, q * 512:(q + 1) * 512],
                                         start=(m == 0), stop=(m == NT // 2 - 1),
                                         perf_mode=DR)

                for m in range(1, NT // 2):
                    ebs[m] = emit_pair_S_exp(p, m)
                    emit_AV(m - 1)
                emit_AV(NT // 2 - 1)

                # flush outT half to SBUF bf16 (frees PSUM for next pass)
                nc.vector.tensor_copy(ot_sb[:, p, :], ot)

            # d = rowsum(x^2): exact diagonal logits (DVE idle mid-loop)
            d_sb = const.tile([128, NT], F32)
            d_scr = const.tile([128, 128], F32)
            for t in range(NT):
                nc.vector.scalar_tensor_tensor(
                    d_scr, x_nat[:, t, :], 1.0, x_nat[:, t, :],
                    MULT, MULT, accum_out=d_sb[:, t:t + 1])
            ed = const.tile([128, NT], F32)               # exp(d*scale+shift)
            nc.scalar.activation(ed, d_sb, EXP, bias=shift_b, scale=SCALE)

            # ---- epilogue: y[i, o] = (outT.T[i, o] + ed[i]*z~[i, o]) / r[i]
            r01 = const.tile([128, NT], F32)
            r23 = const.tile([128, NT], F32)
            rfull = const.tile([128, NT], F32)
            nc.vector.tensor_tensor(r01, r_parts[:, 0, :], r_parts[:, 1, :], ADD)
            nc.vector.tensor_tensor(r23, r_parts[:, 2, :], r_parts[:, 3, :], ADD)
            nc.vector.tensor_tensor(r01, r01, r23, ADD)
            nc.vector.tensor_tensor(rfull, r01, ed, ADD)
            rinv = const.tile([128, NT], F32)
            nc.vector.reciprocal(rinv, rfull)
            er = const.tile([128, NT], F32)               # ed / r
            nc.vector.tensor_tensor(er, ed, rinv, MULT)

            if debug:
                nc.sync.dma_start(dbg_r, r_parts)
                nc.sync.dma_start(dbg_ed, ed)
                nc.sync.dma_start(dbg_rinv, rinv)
                dotf = const.tile([128, NPASS, WPASS], F32, name="dotf")
                nc.vector.tensor_copy(dotf, ot_sb)
                nc.sync.dma_start(dbg_ot, dotf)
                nc.sync.dma_start(dbg_d, d_sb)

            o_view = o_d.rearrange("(t p) c -> p t c", p=128)
            for g in range(NT // 4):
                y4 = ywork.tile([128, 4, 128], F32, name="y4", tag="y4")
                for u in range(4):
                    t = g * 4 + u
                    p = t * 128 // WPASS
                    col = t * 128 - p * WPASS
                    tp_ps = sps.tile([128, 128], BF16, name="tp_ps", tag="sps")
                    nc.tensor.transpose(tp_ps, ot_sb[:, p, col:col + 128], ident)
                    t0 = ywork.tile([128, 128], F32, name="t0", tag="t0")
                    nc.scalar.activation(t0, tp_ps, COPY, scale=rinv[:, t:t + 1])
                    nc.vector.scalar_tensor_tensor(
                        y4[:, u, :], z_bf[:, t, :], er[:, t:t + 1], t0, MULT, ADD)
                nc.sync.dma_start(o_view[:, g * 4:(g + 1) * 4, :], y4)

    nc.compile()
    return nc


_NC_CACHE = {}


def _get_nc():
    if "nc" not in _NC_CACHE:
        _NC_CACHE["nc"] = _build()
    return _NC_CACHE["nc"]


def kernel(x, W, b, _trace=False):
    """x: [8, 4096, 128] f32, W: [128, 128] f32, b: [128] f32 -> [8, 4096, 128] f32."""
    nc = _get_nc()
    x = np.ascontiguousarray(np.asarray(x, dtype=np.float32))
    W = np.ascontiguousarray(np.asarray(W, dtype=np.float32))
    b = np.ascontiguousarray(np.asarray(b, dtype=np.float32))
    in_maps = [{"x": x[i], "W": W, "b": b} for i in range(B)]
    res = bass_utils.run_bass_kernel_spmd(nc, in_maps, core_ids=list(range(B)),
                                          trace=_trace)
    out = np.stack([r["out"] for r in res.results]).astype(np.float32)
    if _trace:
        return out, res
    return out


# revision 11
# speedup vs baseline: 1.2157x; 1.2157x over previous
"""Trainium2 Bass kernel for NonLinearSelfAttention.

Computes, per batch b (one batch per NeuronCore, 8 cores):
    S    = x_b @ x_b.T * C**-0.5          [N, N]
    P    = softmax(S, axis=-1)
    out  = (P @ x_b) @ W.T + bias         [N, OUT]

Per-core algorithm (N=4096, C=128), baseline-derived structure:
  - E = exp(scale*S + shift) is symmetric, so the tile computed in [j, i]
    layout is directly the lhsT needed by the E@V matmul for output block i —
    no transposes in the main loop.
  - The Linear folds entirely through the attention:
    y = (E @ [z~ | 1]) / r with z~ = x @ W.T + bias, because softmax rows
    sum to 1 (P (z + 1 bias^T) = P x W.T + bias exactly).  The ones column
    produces the softmax row sums r in per-partition layout for free.
  - E tiles are stored fp8e5 (after the exp): the AV lhsT loads then use the
    fp8 Fast-Weight-Load path (4 weights/cycle), roughly halving the
    LDWEIGHTS cost that bounds the FD=129 AV matmuls.  e5m2's 57344 max
    cannot overflow for off-diagonal logits; its 2-bit-mantissa noise
    averages out over the 4096-term softmax sums.  The moving z~ stays bf16
    (mixed fp8 x bf16 matmul runs at bf16 speed).
  - The diagonal (logits ~ sqrt(C)) would dwarf fp8: it is masked before the
    exp by accumulating -30000*I into the diagonal 128x128 S block with one
    extra identity matmul, and the exact diagonal term
    ed_i * z~_i (ed = exp(|x_i|^2*scale + shift)) is re-added per i-block in
    the epilogue; r likewise gets ed added before the reciprocal.
"""
import numpy as np

import concourse.bass as bass
import concourse.tile as tile
from concourse.masks import make_identity
from concourse import bacc, mybir
from concourse import bass_utils

B = 8          # batches = cores
N = 4096       # sequence length
C = 128        # feature dim
OUT = 128      # linear out dim
NT = N // 128  # 32 j-tiles
QW = 512       # i-columns processed per quad-block
NQ = N // QW   # 8 quad blocks
SCALE = float(C) ** -0.5
SHIFT = -1.5   # exp bias; cancels in softmax
BIG = 30000.0  # diagonal mask

F32 = mybir.dt.float32
BF16 = mybir.dt.bfloat16
F8E5 = mybir.dt.float8e5
EXP = mybir.ActivationFunctionType.Exp
MULT = mybir.AluOpType.mult
ADD = mybir.AluOpType.add


def _build():
    nc = bacc.Bacc("TRN2", target_bir_lowering=False, debug=False, num_devices=B)
    x_d = nc.dram_tensor("x", [N, C], F32, kind="ExternalInput").ap()
    w_d = nc.dram_tensor("W", [OUT, C], F32, kind="ExternalInput").ap()
    b_d = nc.dram_tensor("b", [OUT], F32, kind="ExternalInput").ap()
    o_d = nc.dram_tensor("out", [N, OUT], F32, kind="ExternalOutput").ap()

    with tile.TileContext(nc) as tc:
        with tc.tile_pool(name="const", bufs=1) as const, \
             tc.tile_pool(name="bwork", bufs=6) as bwork, \
             tc.tile_pool(name="ywork", bufs=2) as ywork, \
             tc.tile_pool(name="ps_work", bufs=2, space="PSUM") as ps_work, \
             tc.tile_pool(name="ps_acc", bufs=2, space="PSUM") as ps_acc:

            # ---- setup ----
            x_nat = const.tile([128, NT, 128], F32)       # x tiles [j in tile, c]
            x_view = x_d.rearrange("(t p) c -> p t c", p=128)
            bounds = [0, 4, 8, 16, 24, NT]
            for lo, hi in zip(bounds, bounds[1:]):
                nc.sync.dma_start(x_nat[:, lo:hi, :], x_view[:, lo:hi, :])

            w_sb = const.tile([128, 128], F32)            # W [o, c]
            nc.sync.dma_start(w_sb, w_d)
            bias_bc = const.tile([128, 128], F32)         # bias broadcast
            nc.sync.dma_start(bias_bc, bass.AP(tensor=b_d.tensor, offset=b_d.offset,
                                               ap=[[0, 128]] + b_d.ap))

            # cast to bf16, then PE-transpose
            x_bf = const.tile([128, NT, 128], BF16)
            for lo, hi in zip(bounds, bounds[1:]):
                nc.vector.tensor_copy(x_bf[:, lo:hi, :], x_nat[:, lo:hi, :])
            w_bf = const.tile([128, 128], BF16)
            nc.vector.tensor_copy(w_bf, w_sb)

            ident = const.tile([128, 128], BF16)
            make_identity(nc, ident)
            identneg = const.tile([128, 128], BF16)       # -BIG * I
            nc.scalar.mul(identneg, ident, -BIG)
            shift_b = const.tile([128, 1], F32)
            nc.vector.memset(shift_b, SHIFT)

            xT = const.tile([128, N], BF16)               # [c, n]

            def emit_xT_group(g):
                t_ps = ps_work.tile([128, 512], BF16, name="t_ps", tag="pswork")
                for u in range(4):
                    nc.tensor.transpose(t_ps[:, u * 128:(u + 1) * 128],
                                        x_bf[:, g * 4 + u, :], ident)
                nc.vector.tensor_copy(xT[:, g * 512:(g + 1) * 512], t_ps)

            xT_state = {"emitted": 0}

            def ensure_xT(j_hi):
                need = min(NT // 4, max(1, (j_hi + 3) // 4))
                while xT_state["emitted"] < need:
                    emit_xT_group(xT_state["emitted"])
                    xT_state["emitted"] += 1

            ensure_xT(4)  # group 0: quad 0's rhs columns
            wt_ps = ps_work.tile([128, 512], BF16, name="t_ps", tag="pswork")
            nc.tensor.transpose(wt_ps[:, 0:128], w_bf, ident)
            wT = const.tile([128, 128], BF16)             # wT[c, o] = W[o, c]
            nc.vector.tensor_copy(wT, wt_ps[:, 0:128])

            # z~ = [x @ W.T + bias | 1]  (bf16), tiled [j within tile, 129]
            zt = const.tile([128, NT, 129], BF16)
            nc.vector.memset(zt[:, :, 128], 1.0)

            def emit_z_group(g):
                z_ps = ps_work.tile([128, 512], F32, name="z_ps", tag="pswork")
                for u in range(4):
                    j = g * 4 + u
                    nc.tensor.matmul(z_ps[:, u * 128:(u + 1) * 128],
                                     xT[:, j * 128:(j + 1) * 128], wT,
                                     start=True, stop=True)
                zv = z_ps.rearrange("p (j c) -> p j c", c=128)
                for u in range(4):
                    nc.vector.tensor_tensor(zt[:, g * 4 + u, 0:128],
                                            zv[:, u, :], bias_bc, ADD)

            z_state = {"emitted": 0}

            def ensure_z(j_hi):
                need = min(NT // 4, (j_hi + 3) // 4)
                while z_state["emitted"] < need:
                    emit_z_group(z_state["emitted"])
                    z_state["emitted"] += 1

            zeros128 = const.tile([128, 128], BF16)
            nc.vector.memset(zeros128, 0.0)
            dummy258 = const.tile([128, 258], BF16)
            nc.vector.memset(dummy258, 0.0)

            # prefetch a couple of xT/z groups so quad 0's pipeline starts deep
            ensure_xT(8)
            ensure_z(4)

            # d = rowsum(x^2): exact diagonal logits; ed = exp(d*scale+shift)
            d_sb = const.tile([128, NT], F32)
            d_scr = const.tile([128, 128], F32)
            for t in range(NT):
                nc.vector.scalar_tensor_tensor(
                    d_scr, x_nat[:, t, :], 1.0, x_nat[:, t, :],
                    MULT, MULT, accum_out=d_sb[:, t:t + 1])
            ed = const.tile([128, NT], F32)
            nc.scalar.activation(ed, d_sb, EXP, bias=shift_b, scale=SCALE)

            # ---- main loop ----
            # exp tiles span up to 3 PSUM banks (j-block groups of 3).  The
            # four acc accumulators pack two-per-bank: a zero matmul opens the
            # bank's accumulation group, then every AV matmul accumulates with
            # start=False.  S-matmuls are emitted one group AHEAD of the AVs.
            JG = [2] + [3] * 10       # j-block group sizes per quad (sum=32)
            NB = QW // 128            # i-blocks per quad (4)
            groups = []
            for q in range(NQ):
                jb = 0
                for hi, gsz in enumerate(JG):
                    groups.append((q, jb, gsz, hi))
                    jb += gsz

            s_tiles = {}

            def emit_S(idx):
                q, jb, gsz, hi = groups[idx]
                ensure_xT(jb + gsz)
                s_ps = ps_work.tile([128, QW * gsz], F32, name="s_ps",
                                    tag="pswork")
                for u in range(gsz):
                    j = jb + u
                    diag = q * 4 <= j < q * 4 + 4   # j-block inside this quad
                    nc.tensor.matmul(s_ps[:, u * QW:(u + 1) * QW],
                                     xT[:, j * 128:(j + 1) * 128],
                                     xT[:, q * QW:(q + 1) * QW],
                                     start=True, stop=True)
                    if diag:
                        off = u * QW + (j - q * 4) * 128
                        nc.tensor.matmul(s_ps[:, off:off + 128],
                                         ident, identneg,
                                         start=False, stop=True,
                                         skip_group_check=True)
                s_tiles[idx] = s_ps

            emit_S(0)
            acc = None
            acc_slice = None
            for idx, (q, jb, gsz, hi) in enumerate(groups):
                if hi == 0:
                    acc = [ps_acc.tile([128, 258], F32, name=f"acc{p}",
                                       tag="acc")
                           for p in range(NB // 2)]

                    def acc_slice(k, w=129, _acc=acc):
                        return _acc[k // 2][:, (k % 2) * 129:(k % 2) * 129 + w]

                if idx + 1 < len(groups):
                    emit_S(idx + 1)
                s_ps = s_tiles.pop(idx)
                b_sb = bwork.tile([128, QW * gsz], F8E5, name="b_sb",
                                  tag="b_sb")
                nc.scalar.activation(b_sb, s_ps, EXP, bias=shift_b, scale=SCALE)
                ensure_z(jb + gsz)
                if hi == 0:
                    for pr in range(NB // 2):
                        nc.tensor.matmul(acc[pr], zeros128, dummy258,
                                         start=True, stop=False,
                                         skip_group_check=True)
                for u in range(gsz):
                    j = jb + u
                    for k in range(NB):
                        nc.tensor.matmul(
                            acc_slice(k),
                            b_sb[:, u * QW + k * 128:u * QW + (k + 1) * 128],
                            zt[:, j, :], start=False, stop=(j == NT - 1),
                            skip_group_check=True)
                if hi != len(JG) - 1:
                    continue
                # epilogue: y = (acc[:, :128] + ed*z~) / (acc[:, 128] + ed)
                y4 = ywork.tile([128, NB, 128], F32, name="y4", tag="y4")
                for k in range(NB):
                    t = q * NB + k
                    radd = ywork.tile([128, 1], F32, name="radd", tag="radd")
                    nc.vector.tensor_tensor(radd, acc_slice(k, 129)[:, 128:129],
                                            ed[:, t:t + 1], ADD)
                    rinv = ywork.tile([128, 1], F32, name="rinv", tag="rinv")
                    nc.vector.reciprocal(rinv, radd)
                    tband = ywork.tile([128, 128], F32, name="tband", tag="tband")
                    nc.vector.scalar_tensor_tensor(
                        tband, zt[:, t, 0:128], ed[:, t:t + 1],
                        acc_slice(k, 128), MULT, ADD)
                    nc.vector.tensor_scalar(y4[:, k, :], tband, rinv, None, MULT)
                o_view = o_d.rearrange("(t p) c -> p t c", p=128)
                nc.sync.dma_start(o_view[:, q * NB:(q + 1) * NB, :], y4)

    nc.compile()
    return nc


_NC_CACHE = {}


def _get_nc():
    if "nc" not in _NC_CACHE:
        _NC_CACHE["nc"] = _build()
    return _NC_CACHE["nc"]


def kernel(x, W, b, _trace=False):
    """x: [8, 4096, 128] f32, W: [128, 128] f32, b: [128] f32 -> [8, 4096, 128] f32."""
    nc = _get_nc()
    x = np.ascontiguousarray(np.asarray(x, dtype=np.float32))
    W = np.ascontiguousarray(np.asarray(W, dtype=np.float32))
    b = np.ascontiguousarray(np.asarray(b, dtype=np.float32))
    in_maps = [{"x": x[i], "W": W, "b": b} for i in range(B)]
    res = bass_utils.run_bass_kernel_spmd(nc, in_maps, core_ids=list(range(B)),
                                          trace=_trace)
    out = np.stack([r["out"] for r in res.results]).astype(np.float32)
    if _trace:
        return out, res
    return out


# revision 12
# speedup vs baseline: 1.3303x; 1.0943x over previous
"""Trainium2 Bass kernel for NonLinearSelfAttention.

Computes, per batch b (one batch per NeuronCore, 8 cores):
    S    = x_b @ x_b.T * C**-0.5          [N, N]
    P    = softmax(S, axis=-1)
    out  = (P @ x_b) @ W.T + bias         [N, OUT]

Per-core algorithm (N=4096, C=128), baseline-derived structure:
  - E = exp(scale*S + shift) is symmetric, so the tile computed in [j, i]
    layout is directly the lhsT needed by the E@V matmul for output block i —
    no transposes in the main loop.
  - The Linear folds entirely through the attention:
    y = (E @ [z~ | 1]) / r with z~ = x @ W.T + bias, because softmax rows
    sum to 1 (P (z + 1 bias^T) = P x W.T + bias exactly).  The ones column
    produces the softmax row sums r in per-partition layout for free.
  - E tiles are stored fp8e5 (after the exp): the AV lhsT loads then use the
    fp8 Fast-Weight-Load path (4 weights/cycle), roughly halving the
    LDWEIGHTS cost that bounds the FD=129 AV matmuls.  e5m2's 57344 max
    cannot overflow for off-diagonal logits; its 2-bit-mantissa noise
    averages out over the 4096-term softmax sums.  The moving z~ stays bf16
    (mixed fp8 x bf16 matmul runs at bf16 speed).
  - The diagonal (logits ~ sqrt(C)) would dwarf fp8: it is masked before the
    exp by accumulating -30000*I into the diagonal 128x128 S block with one
    extra identity matmul, and the exact diagonal term
    ed_i * z~_i (ed = exp(|x_i|^2*scale + shift)) is re-added per i-block in
    the epilogue; r likewise gets ed added before the reciprocal.
"""
import numpy as np

import concourse.bass as bass
import concourse.tile as tile
from concourse.masks import make_identity
from concourse import bacc, mybir
from concourse import bass_utils

B = 8          # batches = cores
N = 4096       # sequence length
C = 128        # feature dim
OUT = 128      # linear out dim
NT = N // 128  # 32 j-tiles
QW = 512       # i-columns processed per quad-block
NQ = N // QW   # 8 quad blocks
SCALE = float(C) ** -0.5
SHIFT = -1.5   # exp bias; cancels in softmax
BIG = 30000.0  # diagonal mask

F32 = mybir.dt.float32
BF16 = mybir.dt.bfloat16
F8E5 = mybir.dt.float8e5
EXP = mybir.ActivationFunctionType.Exp
MULT = mybir.AluOpType.mult
ADD = mybir.AluOpType.add


def _build():
    nc = bacc.Bacc("TRN2", target_bir_lowering=False, debug=False, num_devices=B)
    x_d = nc.dram_tensor("x", [N, C], F32, kind="ExternalInput").ap()
    w_d = nc.dram_tensor("W", [OUT, C], F32, kind="ExternalInput").ap()
    b_d = nc.dram_tensor("b", [OUT], F32, kind="ExternalInput").ap()
    o_d = nc.dram_tensor("out", [N, OUT], F32, kind="ExternalOutput").ap()

    with tile.TileContext(nc) as tc:
        with tc.tile_pool(name="const", bufs=1) as const, \
             tc.tile_pool(name="bwork", bufs=6) as bwork, \
             tc.tile_pool(name="ywork", bufs=2) as ywork, \
             tc.tile_pool(name="ps_work", bufs=3, space="PSUM") as ps_work, \
             tc.tile_pool(name="ps_acc", bufs=2, space="PSUM") as ps_acc:

            # ---- setup ----
            x_nat = const.tile([128, NT, 128], F32)       # x tiles [j in tile, c]
            x_view = x_d.rearrange("(t p) c -> p t c", p=128)
            bounds = [0, 4, 8, 16, 24, NT]
            for lo, hi in zip(bounds, bounds[1:]):
                nc.sync.dma_start(x_nat[:, lo:hi, :], x_view[:, lo:hi, :])

            w_sb = const.tile([128, 128], F32)            # W [o, c]
            nc.sync.dma_start(w_sb, w_d)
            bias_bc = const.tile([128, 128], F32)         # bias broadcast
            nc.sync.dma_start(bias_bc, bass.AP(tensor=b_d.tensor, offset=b_d.offset,
                                               ap=[[0, 128]] + b_d.ap))

            # cast to bf16, then PE-transpose
            x_bf = const.tile([128, NT, 128], BF16)
            for lo, hi in zip(bounds, bounds[1:]):
                nc.vector.tensor_copy(x_bf[:, lo:hi, :], x_nat[:, lo:hi, :])
            w_bf = const.tile([128, 128], BF16)
            nc.vector.tensor_copy(w_bf, w_sb)

            ident = const.tile([128, 128], BF16)
            make_identity(nc, ident)
            identneg = const.tile([128, 128], BF16)       # -BIG * I
            nc.scalar.mul(identneg, ident, -BIG)
            shift_b = const.tile([128, 1], F32)
            nc.vector.memset(shift_b, SHIFT)

            xT = const.tile([128, N], BF16)               # [c, n]

            def emit_xT_group(g):
                t_ps = ps_work.tile([128, 512], BF16, name="t_ps", tag="pswork")
                for u in range(4):
                    nc.tensor.transpose(t_ps[:, u * 128:(u + 1) * 128],
                                        x_bf[:, g * 4 + u, :], ident)
                nc.vector.tensor_copy(xT[:, g * 512:(g + 1) * 512], t_ps)

            xT_state = {"emitted": 0}

            def ensure_xT(j_hi):
                need = min(NT // 4, max(1, (j_hi + 3) // 4))
                while xT_state["emitted"] < need:
                    emit_xT_group(xT_state["emitted"])
                    xT_state["emitted"] += 1

            ensure_xT(4)  # group 0: quad 0's rhs columns
            wt_ps = ps_work.tile([128, 512], BF16, name="t_ps", tag="pswork")
            nc.tensor.transpose(wt_ps[:, 0:128], w_bf, ident)
            wT = const.tile([128, 128], BF16)             # wT[c, o] = W[o, c]
            nc.vector.tensor_copy(wT, wt_ps[:, 0:128])

            # z~ = [x @ W.T + bias | 1]  (bf16), tiled [j within tile, 129]
            zt = const.tile([128, NT, 129], BF16)
            nc.vector.memset(zt[:, :, 128], 1.0)

            def emit_z_group(g):
                z_ps = ps_work.tile([128, 512], F32, name="z_ps", tag="pswork")
                for u in range(4):
                    j = g * 4 + u
                    nc.tensor.matmul(z_ps[:, u * 128:(u + 1) * 128],
                                     xT[:, j * 128:(j + 1) * 128], wT,
                                     start=True, stop=True)
                zv = z_ps.rearrange("p (j c) -> p j c", c=128)
                for u in range(4):
                    nc.vector.tensor_tensor(zt[:, g * 4 + u, 0:128],
                                            zv[:, u, :], bias_bc, ADD)

            z_state = {"emitted": 0}

            def ensure_z(j_hi):
                need = min(NT // 4, (j_hi + 3) // 4)
                while z_state["emitted"] < need:
                    emit_z_group(z_state["emitted"])
                    z_state["emitted"] += 1

            zeros128 = const.tile([128, 128], BF16)
            nc.vector.memset(zeros128, 0.0)
            dummy258 = const.tile([128, 258], BF16)
            nc.vector.memset(dummy258, 0.0)

            # prefetch a couple of xT/z groups so quad 0's pipeline starts deep
            ensure_xT(8)
            ensure_z(4)

            # d = rowsum(x^2): exact diagonal logits; ed = exp(d*scale+shift)
            d_sb = const.tile([128, NT], F32)
            d_scr = const.tile([128, 128], F32)
            for t in range(NT):
                nc.vector.scalar_tensor_tensor(
                    d_scr, x_nat[:, t, :], 1.0, x_nat[:, t, :],
                    MULT, MULT, accum_out=d_sb[:, t:t + 1])
            ed = const.tile([128, NT], F32)
            nc.scalar.activation(ed, d_sb, EXP, bias=shift_b, scale=SCALE)

            # ---- main loop ----
            # exp tiles span up to 3 PSUM banks (j-block groups of 3).  The
            # four acc accumulators pack two-per-bank: a zero matmul opens the
            # bank's accumulation group, then every AV matmul accumulates with
            # start=False.  S-matmuls are emitted one group AHEAD of the AVs.
            JG = [2] * 16             # j-block group sizes per quad (sum=32)
            NB = QW // 128            # i-blocks per quad (4)
            groups = []
            for q in range(NQ):
                jb = 0
                for hi, gsz in enumerate(JG):
                    groups.append((q, jb, gsz, hi))
                    jb += gsz

            s_tiles = {}

            def emit_S(idx):
                q, jb, gsz, hi = groups[idx]
                ensure_xT(jb + gsz)
                s_ps = ps_work.tile([128, QW * gsz], F32, name="s_ps",
                                    tag="pswork")
                for u in range(gsz):
                    j = jb + u
                    diag = q * 4 <= j < q * 4 + 4   # j-block inside this quad
                    nc.tensor.matmul(s_ps[:, u * QW:(u + 1) * QW],
                                     xT[:, j * 128:(j + 1) * 128],
                                     xT[:, q * QW:(q + 1) * QW],
                                     start=True, stop=True)
                    if diag:
                        off = u * QW + (j - q * 4) * 128
                        nc.tensor.matmul(s_ps[:, off:off + 128],
                                         ident, identneg,
                                         start=False, stop=True,
                                         skip_group_check=True)
                s_tiles[idx] = s_ps

            emit_S(0)
            emit_S(1)
            acc = None
            acc_slice = None
            for idx, (q, jb, gsz, hi) in enumerate(groups):
                if hi == 0:
                    acc = [ps_acc.tile([128, 258], F32, name=f"acc{p}",
                                       tag="acc")
                           for p in range(NB // 2)]

                    def acc_slice(k, w=129, _acc=acc):
                        return _acc[k // 2][:, (k % 2) * 129:(k % 2) * 129 + w]

                if idx + 2 < len(groups):
                    emit_S(idx + 2)
                s_ps = s_tiles.pop(idx)
                b_sb = bwork.tile([128, QW * gsz], F8E5, name="b_sb",
                                  tag="b_sb")
                nc.scalar.activation(b_sb, s_ps, EXP, bias=shift_b, scale=SCALE)
                ensure_z(jb + gsz)
                if hi == 0:
                    for pr in range(NB // 2):
                        nc.tensor.matmul(acc[pr], zeros128, dummy258,
                                         start=True, stop=False,
                                         skip_group_check=True)
                for u in range(gsz):
                    j = jb + u
                    for k in range(NB):
                        nc.tensor.matmul(
                            acc_slice(k),
                            b_sb[:, u * QW + k * 128:u * QW + (k + 1) * 128],
                            zt[:, j, :], start=False, stop=(j == NT - 1),
                            skip_group_check=True)
                if hi != len(JG) - 1:
                    continue
                # epilogue: y = (acc[:, :128] + ed*z~) / (acc[:, 128] + ed)
                y4 = ywork.tile([128, NB, 128], F32, name="y4", tag="y4")
                for k in range(NB):
                    t = q * NB + k
                    radd = ywork.tile([128, 1], F32, name="radd", tag="radd")
                    nc.vector.tensor_tensor(radd, acc_slice(k, 129)[:, 128:129],
                                            ed[:, t:t + 1], ADD)
                    rinv = ywork.tile([128, 1], F32, name="rinv", tag="rinv")
                    nc.vector.reciprocal(rinv, radd)
                    tband = ywork.tile([128, 128], F32, name="tband", tag="tband")
                    nc.vector.scalar_tensor_tensor(
                        tband, zt[:, t, 0:128], ed[:, t:t + 1],
                        acc_slice(k, 128), MULT, ADD)
                    nc.vector.tensor_scalar(y4[:, k, :], tband, rinv, None, MULT)
                o_view = o_d.rearrange("(t p) c -> p t c", p=128)
                nc.sync.dma_start(o_view[:, q * NB:(q + 1) * NB, :], y4)

    nc.compile()
    return nc


_NC_CACHE = {}


def _get_nc():
    if "nc" not in _NC_CACHE:
        _NC_CACHE["nc"] = _build()
    return _NC_CACHE["nc"]


def kernel(x, W, b, _trace=False):
    """x: [8, 4096, 128] f32, W: [128, 128] f32, b: [128] f32 -> [8, 4096, 128] f32."""
    nc = _get_nc()
    x = np.ascontiguousarray(np.asarray(x, dtype=np.float32))
    W = np.ascontiguousarray(np.asarray(W, dtype=np.float32))
    b = np.ascontiguousarray(np.asarray(b, dtype=np.float32))
    in_maps = [{"x": x[i], "W": W, "b": b} for i in range(B)]
    res = bass_utils.run_bass_kernel_spmd(nc, in_maps, core_ids=list(range(B)),
                                          trace=_trace)
    out = np.stack([r["out"] for r in res.results]).astype(np.float32)
    if _trace:
        return out, res
    return out
